# revision 14
# baseline (speedup 1.0000x reference)
# Trainium2 Bass kernel for nn_BertProber (segment_reduce, memory-bound).
#
# Sharding: pure data parallel over the sentence dim N=1024 -> 8 cores x 128
# sentences. Each core processes its 128 review + 128 reply sentences
# independently; no collectives.
#
# Per-core algorithm:
#   For each sentence n the reference computes two weighted token means over
#   the [L=128, H=768] feature tile: a sentence mean (tokens 1..nt) and a
#   probing-topic mean (union of <=5 spans, falling back to the sentence mean
#   when no span is valid). Both are expressible as  w.T @ feat  with a
#   per-sentence weight vector w[L] that already folds in the 1/count scaling
#   and the has_span selection:
#     1. Build all weight vectors on-chip in [n_part, L_free] layout with DVE
#        compare/accumulate ops on an iota constant (span union via
#        difference-of-step-functions: sum_k ge(l,s_k) - ge(l,e_k+1)).
#     2. Transpose to [L, n] via PE and interleave (pt, sent) columns.
#     3. Main loop: DMA feature tiles [128L, 4 sentences x 768H]; per sentence
#        two matmuls (H split 512+256) with lhsT = 32 interleaved weight
#        columns so results land at PSUM partitions {32j, 32j+1} and every
#        PSUM partition is written (M doesn't affect PE cost).
#     4. Drain PSUM -> SBUF staging on the Scalar engine; single-partition
#        DMAs scatter the pt/sent rows to DRAM.
#
# Written in raw bass (manual semaphores, standalone wait instructions): the
# Tile layer's generated code (multi-wait instructions, RANGE_CLEAR cleanup)
# does not compile with this container's walrus build.
import numpy as np

N, L, H, K = 1024, 128, 768, 5
NCORES = 8
NS = N // NCORES   # sentences per core
G8 = 8             # 4-sentence groups per staging supertile (32 sentences)
NBUF = 4           # feature-tile double buffering depth
NGRP = NS // 4     # groups per kind
NSG = NGRP // G8   # supertiles per kind

# "float32" = exact, 4 PE cycles/row. "float32r" = reduced precision, 1 cyc/row.
MM_DTYPE = "float32"

_CACHE = {}


def _build_nc():
    import concourse.bass as bass
    import concourse.mybir as mybir
    from contextlib import ExitStack

    f32 = mybir.dt.float32
    i32 = mybir.dt.int32
    mmdt = getattr(mybir.dt, MM_DTYPE)
    Alu = mybir.AluOpType

    nc = bass.Bass(trn_type="TRN2")

    kinds = ("rv", "rp")
    ins = {}
    outs = {}
    for kind in kinds:
        ins[f"{kind}_feat"] = nc.dram_tensor(
            f"{kind}_feat", [NS, L, H], mmdt, kind="ExternalInput")
        ins[f"{kind}_nt"] = nc.dram_tensor(
            f"{kind}_nt", [NS], i32, kind="ExternalInput")
        ins[f"{kind}_ss"] = nc.dram_tensor(
            f"{kind}_ss", [NS, K], i32, kind="ExternalInput")
        ins[f"{kind}_se"] = nc.dram_tensor(
            f"{kind}_se", [NS, K], i32, kind="ExternalInput")
        outs[f"{kind}_pt"] = nc.dram_tensor(
            f"{kind}_pt", [NS, H], f32, kind="ExternalOutput")
        outs[f"{kind}_sent"] = nc.dram_tensor(
            f"{kind}_sent", [NS, H], f32, kind="ExternalOutput")

    # Compile-time constants.
    iota_row = np.tile(np.arange(L, dtype=np.float32)[None, :], (128, 1))
    iota_sent_np = iota_row.copy()
    iota_sent_np[:, 0] = 1000.0  # position 0 ([CLS]) never in the sentence mask
    ident_np = np.eye(128, dtype=np.float32)
    iota_span_d = nc.inline_tensor(iota_row, name="iota_span_c")
    iota_sent_d = nc.inline_tensor(iota_sent_np, name="iota_sent_c")
    ident_d = nc.inline_tensor(ident_np, name="ident_c")

    with ExitStack() as ctx:
        def sb(name, shape, dt):
            return ctx.enter_context(nc.sbuf_tensor(name, shape, dt))

        def ps(name, shape, dt):
            return ctx.enter_context(nc.psum_tensor(name, shape, dt))

        def sem(name):
            return ctx.enter_context(nc.semaphore(name))

        iota_span = sb("iota_span", [128, L], f32)
        iota_sent = sb("iota_sent", [128, L], f32)
        ident = sb("ident", [128, 128], f32)
        # per-kind weight-phase tiles
        wtiles = {}
        for kind in kinds:
            wtiles[kind] = {
                "nt_i": sb(f"nt_i_{kind}", [NS, 1], i32),
                "ss_i": sb(f"ss_i_{kind}", [NS, K], i32),
                "se_i": sb(f"se_i_{kind}", [NS, K], i32),
                "ntf": sb(f"ntf_{kind}", [NS, 1], f32),
                "ssf": sb(f"ssf_{kind}", [NS, K], f32),
                "sep1": sb(f"sep1_{kind}", [NS, K], f32),
                "sep1f": sb(f"sep1f_{kind}", [NS, K], f32),
                "w_pt": sb(f"w_pt_{kind}", [NS, L], f32),
                "w_sent": sb(f"w_sent_{kind}", [NS, L], f32),
                "wT": sb(f"wT_{kind}", [128, 2 * NS + 32], mmdt),
            }
        # shared DVE scratch
        sm = sb("sm", [NS, L], f32)
        spm = sb("spm", [NS, L], f32)
        tmp = sb("tmp", [NS, L], f32)
        cnt_s = sb("cnt_s", [NS, 1], f32)
        cnt_p = sb("cnt_p", [NS, 1], f32)
        has = sb("has", [NS, 1], f32)
        cntp_c = sb("cntp_c", [NS, 1], f32)
        recip_p = sb("recip_p", [NS, 1], f32)
        recip_s = sb("recip_s", [NS, 1], f32)
        rp_sel = sb("rp_sel", [NS, 1], f32)
        nh = sb("nh", [NS, 1], f32)
        rs_sel = sb("rs_sel", [NS, 1], f32)

        ft = [sb(f"ft{i}", [128, 4, H], mmdt) for i in range(NBUF)]
        stage = [sb(f"stage{i}", [128, G8, H], f32) for i in range(2)]
        psA = [ps(f"psA{i}", [128, 512], f32) for i in range(2)]
        psB = [ps(f"psB{i}", [128, 512], f32) for i in range(2)]

        const_sem = sem("const_sem")   # iota/ident DMAs         (SP -> DVE/PE)
        wdma_sem = sem("wdma_sem")    # nt/ss/se DMAs           (SP -> DVE)
        dve_sem = sem("dve_sem")     # per-kind weights ready  (DVE -> PE)
        pe_w_sem = sem("pe_w_sem")    # per-kind transposes     (PE -> ACT)
        wact_sem = sem("wact_sem")    # per-kind wT interleave  (ACT -> PE)
        ft_sem = [sem(f"ft_sem{i}") for i in range(NBUF)]  # feat DMAs (SP -> PE)
        pe_grp = sem("pe_grp")      # per-group matmuls done  (PE -> ACT, SP)
        act_grp = sem("act_grp")     # per-group drains done   (ACT -> PE)
        odma_sem = [sem(f"odma_sem{i}") for i in range(2)]  # out DMAs per stage buf
        dve_chain = sem("dve_chain")  # same-engine RAW ordering on DVE
        act_chain = sem("act_chain")  # ACT copy -> ACT-issued DMA ordering

        with nc.Block() as block:

            @block.sync
            def _(sync):
                sync.dma_start(out=iota_span[:], in_=iota_span_d[:, :]).then_inc(const_sem, 16)
                sync.dma_start(out=iota_sent[:], in_=iota_sent_d[:, :]).then_inc(const_sem, 16)
                sync.dma_start(out=ident[:], in_=ident_d[:, :]).then_inc(const_sem, 16)
                for kind in kinds:
                    w = wtiles[kind]
                    sync.dma_start(out=w["nt_i"][:], in_=ins[f"{kind}_nt"][:].unsqueeze(1)).then_inc(wdma_sem, 16)
                    sync.dma_start(out=w["ss_i"][:], in_=ins[f"{kind}_ss"][:, :]).then_inc(wdma_sem, 16)
                    sync.dma_start(out=w["se_i"][:], in_=ins[f"{kind}_se"][:, :]).then_inc(wdma_sem, 16)
                # main loop: feature tile loads
                for kidx, kind in enumerate(kinds):
                    feat = ins[f"{kind}_feat"]
                    for g in range(NGRP):
                        gg = kidx * NGRP + g
                        if gg >= NBUF:
                            sync.wait_ge(pe_grp, gg - NBUF + 1)
                        sync.dma_start(
                            out=ft[gg % NBUF][:],
                            in_=feat[4 * g:4 * g + 4, :, :].transpose([1, 0, 2]),
                        ).then_inc(ft_sem[gg % NBUF], 16)

            @block.vector
            def _(vector):
                # The DVE pipeline has no interlock for back-to-back RAW, so
                # chain every op through a self-semaphore.
                nv = [0]

                def dv(res):
                    res.then_inc(dve_chain, 1)
                    nv[0] += 1

                def dw():
                    if nv[0]:
                        vector.wait_ge(dve_chain, nv[0])

                vector.wait_ge(const_sem, 48)
                for kidx, kind in enumerate(kinds):
                    w = wtiles[kind]
                    if kidx == 0:
                        vector.wait_ge(wdma_sem, 96)
                    dv(vector.tensor_copy(out=w["ntf"][:], in_=w["nt_i"][:]))
                    dv(vector.tensor_copy(out=w["ssf"][:], in_=w["ss_i"][:]))
                    dv(vector.tensor_scalar(
                        out=w["sep1"][:], in0=w["se_i"][:], scalar1=1,
                        scalar2=None, op0=Alu.add))
                    dw()
                    dv(vector.tensor_copy(out=w["sep1f"][:], in_=w["sep1"][:]))
                    # sentence mask + count
                    dw()
                    dv(vector.tensor_scalar(
                        out=sm[:], in0=iota_sent[:], scalar1=w["ntf"][:],
                        scalar2=0.0, op0=Alu.is_le, op1=Alu.add,
                        accum_out=cnt_s[:]))
                    # span union mask: sum_k [ge(l, s_k) - ge(l, e_k + 1)]
                    dv(vector.tensor_scalar(
                        out=spm[:], in0=iota_span[:], scalar1=w["ssf"][:, 0:1],
                        scalar2=None, op0=Alu.is_ge))
                    for k in range(1, K):
                        dw()
                        dv(vector.tensor_scalar(
                            out=tmp[:], in0=iota_span[:],
                            scalar1=w["ssf"][:, k:k + 1],
                            scalar2=None, op0=Alu.is_ge))
                        dw()
                        dv(vector.tensor_tensor(out=spm[:], in0=spm[:], in1=tmp[:], op=Alu.add))
                    for k in range(K):
                        dw()
                        dv(vector.tensor_scalar(
                            out=tmp[:], in0=iota_span[:],
                            scalar1=w["sep1f"][:, k:k + 1],
                            scalar2=None, op0=Alu.is_ge))
                        dw()
                        dv(vector.tensor_tensor(out=spm[:], in0=spm[:], in1=tmp[:], op=Alu.subtract))
                    dw()
                    dv(vector.reduce_sum(out=cnt_p[:], in_=spm[:], axis=mybir.AxisListType.X))
                    dw()
                    dv(vector.tensor_scalar(
                        out=has[:], in0=cnt_p[:], scalar1=1.0, scalar2=None, op0=Alu.is_ge))
                    dv(vector.tensor_scalar(
                        out=cntp_c[:], in0=cnt_p[:], scalar1=1.0, scalar2=None, op0=Alu.max))
                    dw()
                    dv(vector.reciprocal(out=recip_p[:], in_=cntp_c[:]))
                    dv(vector.reciprocal(out=recip_s[:], in_=cnt_s[:]))
                    dw()
                    dv(vector.tensor_tensor(out=rp_sel[:], in0=has[:], in1=recip_p[:], op=Alu.mult))
                    dv(vector.tensor_scalar(
                        out=nh[:], in0=has[:], scalar1=-1.0, scalar2=1.0,
                        op0=Alu.mult, op1=Alu.add))
                    dw()
                    dv(vector.tensor_tensor(out=rs_sel[:], in0=nh[:], in1=recip_s[:], op=Alu.mult))
                    dv(vector.tensor_scalar(
                        out=w["w_sent"][:], in0=sm[:], scalar1=recip_s[:],
                        scalar2=None, op0=Alu.mult))
                    dv(vector.tensor_scalar(
                        out=w["w_pt"][:], in0=spm[:], scalar1=rp_sel[:],
                        scalar2=None, op0=Alu.mult))
                    dw()
                    dv(vector.tensor_scalar(
                        out=tmp[:], in0=sm[:], scalar1=rs_sel[:],
                        scalar2=None, op0=Alu.mult))
                    dw()
                    dv(vector.tensor_tensor(out=w["w_pt"][:], in0=w["w_pt"][:], in1=tmp[:], op=Alu.add))
                    # zero pad columns (so M=32 matmuls can slide past the end)
                    dw()
                    vector.memset(w["wT"][:, 2 * NS:], 0.0).then_inc(dve_sem, 1)

            @block.tensor
            def _(tensor):
                # weight transposes [n, L] -> [L, n], into main-loop PSUM banks
                for kidx, kind in enumerate(kinds):
                    w = wtiles[kind]
                    tensor.wait_ge(dve_sem, kidx + 1)
                    tensor.transpose(psA[kidx][:, 0:128], w["w_pt"][:], ident[:])
                    tensor.transpose(
                        psB[kidx][:, 0:128], w["w_sent"][:], ident[:]
                    ).then_inc(pe_w_sem, 1)
                # main loop
                tensor.wait_ge(wact_sem, 2)
                for kidx, kind in enumerate(kinds):
                    wT = wtiles[kind]["wT"]
                    for g in range(NGRP):
                        gg = kidx * NGRP + g
                        pb = gg % 2
                        tensor.wait_ge(ft_sem[gg % NBUF], 16 * (gg // NBUF + 1))
                        if gg >= 2:
                            tensor.wait_ge(act_grp, gg - 1)
                        for j in range(4):
                            n_ = 4 * g + j
                            lhsT = wT[:, 2 * n_:2 * n_ + 32]
                            tensor.matmul(
                                out=psA[pb][32 * j:32 * j + 32, :], lhsT=lhsT,
                                rhs=ft[gg % NBUF][:, j, 0:512],
                                start=True, stop=True, tile_position=(0, 32 * j))
                            mm = tensor.matmul(
                                out=psB[pb][32 * j:32 * j + 32, 0:256], lhsT=lhsT,
                                rhs=ft[gg % NBUF][:, j, 512:H],
                                start=True, stop=True, tile_position=(0, 32 * j))
                        mm.then_inc(pe_grp, 1)

            @block.scalar
            def _(scalar):
                # wT interleave: even cols = pt weights, odd cols = sent weights
                for kidx, kind in enumerate(kinds):
                    w = wtiles[kind]
                    wT_v = w["wT"][:, 0:2 * NS].rearrange("p (n t) -> p t n", t=2)
                    scalar.wait_ge(pe_w_sem, kidx + 1)
                    scalar.copy(out=wT_v[:, 0, :], in_=psA[kidx][:, 0:128])
                    cp = scalar.copy(out=wT_v[:, 1, :], in_=psB[kidx][:, 0:128])
                    cp.then_inc(wact_sem, 1)
                # main loop: drains + output DMAs
                n_odma = 0
                for kidx, kind in enumerate(kinds):
                    pt_out = outs[f"{kind}_pt"]
                    sent_out = outs[f"{kind}_sent"]
                    for g in range(NGRP):
                        gg = kidx * NGRP + g
                        pb = gg % 2
                        sgg = gg // G8
                        st = stage[sgg % 2]
                        if g % G8 == 0 and sgg >= 2:
                            scalar.wait_ge(odma_sem[sgg % 2], 128 * (sgg // 2))
                        scalar.wait_ge(pe_grp, gg + 1)
                        scalar.copy(out=st[:, g % G8, 0:512], in_=psA[pb][:])
                        cp = scalar.copy(out=st[:, g % G8, 512:H], in_=psB[pb][:, 0:256])
                        cp.then_inc(act_grp, 1)
                        if g % G8 == G8 - 1:
                            # ensure the staging writes retired before the
                            # DMA engines read them
                            scalar.wait_ge(act_grp, gg + 1)
                            base = 4 * G8 * (g // G8)
                            for j in range(4):
                                scalar.dma_start(
                                    out=pt_out[base + j:base + 4 * G8:4, :].unsqueeze(0),
                                    in_=st[32 * j:32 * j + 1, :, :],
                                ).then_inc(odma_sem[sgg % 2], 16)
                                scalar.dma_start(
                                    out=sent_out[base + j:base + 4 * G8:4, :].unsqueeze(0),
                                    in_=st[32 * j + 1:32 * j + 2, :, :],
                                ).then_inc(odma_sem[sgg % 2], 16)
                                n_odma += 2
                total_sg = 2 * NSG
                scalar.wait_ge(odma_sem[0], 128 * ((total_sg + 1) // 2))
                scalar.wait_ge(odma_sem[1], 128 * (total_sg // 2))

    return nc


def _get_nc():
    if "nc" not in _CACHE:
        _CACHE["nc"] = _build_nc()
    return _CACHE["nc"]


def _make_in_maps(inputs):
    def npa(x, dt):
        return np.ascontiguousarray(np.asarray(x), dtype=dt)

    full = {
        "rv_feat": npa(inputs["review_feat"], np.float32),
        "rp_feat": npa(inputs["reply_feat"], np.float32),
        "rv_nt": npa(inputs["review_num_tokens"], np.int32),
        "rp_nt": npa(inputs["reply_num_tokens"], np.int32),
        "rv_ss": npa(inputs["review_span_start"], np.int32),
        "rv_se": npa(inputs["review_span_end"], np.int32),
        "rp_ss": npa(inputs["reply_span_start"], np.int32),
        "rp_se": npa(inputs["reply_span_end"], np.int32),
    }
    in_maps = []
    for c in range(NCORES):
        sl = slice(c * NS, (c + 1) * NS)
        in_maps.append({k: v[sl] for k, v in full.items()})
    return in_maps


def _gather(results):
    def cat(name):
        return np.concatenate([results[c][name] for c in range(NCORES)], axis=0)

    return cat("rv_pt"), cat("rv_sent"), cat("rp_pt"), cat("rp_sent")


def kernel(**inputs):
    from concourse.bass_utils import run_bass_kernel_spmd

    nc = _get_nc()
    in_maps = _make_in_maps(inputs)
    res = run_bass_kernel_spmd(nc, in_maps, list(range(NCORES)))
    return _gather(res.results)


# revision 15
# speedup vs baseline: 450.7765x; 450.7765x over previous
# Trainium2 Bass kernel for nn_BertProber (segment_reduce, memory-bound).
#
# Sharding: pure data parallel over the sentence dim N=1024 -> 8 cores x 128
# sentences. Each core processes its 128 review + 128 reply sentences
# independently; no collectives.
#
# Per-core algorithm:
#   For each sentence n the reference computes two weighted token means over
#   the [L=128, H=768] feature tile: a sentence mean (tokens 1..nt) and a
#   probing-topic mean (union of <=5 spans, falling back to the sentence mean
#   when no span is valid). Both are expressible as  w.T @ feat  with a
#   per-sentence weight vector w[L] that already folds in the 1/count scaling
#   and the has_span selection:
#     1. Build all weight vectors on-chip in [n_part, L_free] layout with DVE
#        compare/accumulate ops on an iota constant (span union via
#        difference-of-step-functions: sum_k ge(l,s_k) - ge(l,e_k+1)).
#     2. Transpose to [L, n] via PE and interleave (pt, sent) columns.
#     3. Main loop: DMA feature tiles [128L, 4 sentences x 768H]; per sentence
#        two matmuls (H split 512+256) with lhsT = 32 interleaved weight
#        columns so results land at PSUM partitions {32j, 32j+1} and every
#        PSUM partition is written (M doesn't affect PE cost).
#     4. Drain PSUM -> SBUF staging on the Scalar engine; single-partition
#        DMAs scatter the pt/sent rows to DRAM.
#
# Written in raw bass (manual semaphores, standalone wait instructions): the
# Tile layer's generated code (multi-wait instructions, RANGE_CLEAR cleanup)
# does not compile with this container's walrus build.
import numpy as np

N, L, H, K = 1024, 128, 768, 5
NCORES = 8
NS = N // NCORES   # sentences per core
G8 = 8             # 4-sentence groups per staging supertile (32 sentences)
NBUF = 4           # feature-tile double buffering depth
NGRP = NS // 4     # groups per kind
NSG = NGRP // G8   # supertiles per kind

# "float32" = exact, 4 PE cycles/row. "float32r" = reduced precision, 1 cyc/row.
MM_DTYPE = "float32"

_CACHE = {}


def _build_nc(repeat=1):
    import concourse.bass as bass
    import concourse.mybir as mybir
    from contextlib import ExitStack

    f32 = mybir.dt.float32
    i32 = mybir.dt.int32
    mmdt = getattr(mybir.dt, MM_DTYPE)
    Alu = mybir.AluOpType

    nc = bass.Bass(trn_type="TRN2")

    kinds = ("rv", "rp")
    ins = {}
    outs = {}
    for kind in kinds:
        ins[f"{kind}_feat"] = nc.dram_tensor(
            f"{kind}_feat", [NS, L, H], mmdt, kind="ExternalInput")
        ins[f"{kind}_nt"] = nc.dram_tensor(
            f"{kind}_nt", [NS], i32, kind="ExternalInput")
        ins[f"{kind}_ss"] = nc.dram_tensor(
            f"{kind}_ss", [NS, K], i32, kind="ExternalInput")
        ins[f"{kind}_se"] = nc.dram_tensor(
            f"{kind}_se", [NS, K], i32, kind="ExternalInput")
        outs[f"{kind}_pt"] = nc.dram_tensor(
            f"{kind}_pt", [NS, H], f32, kind="ExternalOutput")
        outs[f"{kind}_sent"] = nc.dram_tensor(
            f"{kind}_sent", [NS, H], f32, kind="ExternalOutput")

    # Compile-time constants.
    iota_row = np.tile(np.arange(L, dtype=np.float32)[None, :], (128, 1))
    iota_sent_np = iota_row.copy()
    iota_sent_np[:, 0] = 1000.0  # position 0 ([CLS]) never in the sentence mask
    ident_np = np.eye(128, dtype=np.float32)
    iota_span_d = nc.inline_tensor(iota_row, name="iota_span_c")
    iota_sent_d = nc.inline_tensor(iota_sent_np, name="iota_sent_c")
    ident_d = nc.inline_tensor(ident_np, name="ident_c")

    with ExitStack() as ctx:
        def sb(name, shape, dt):
            return ctx.enter_context(nc.sbuf_tensor(name, shape, dt))

        def ps(name, shape, dt):
            return ctx.enter_context(nc.psum_tensor(name, shape, dt))

        def sem(name):
            return ctx.enter_context(nc.semaphore(name))

        iota_span = sb("iota_span", [128, L], f32)
        iota_sent = sb("iota_sent", [128, L], f32)
        ident = sb("ident", [128, 128], f32)
        # per-kind weight-phase tiles
        wtiles = {}
        for kind in kinds:
            wtiles[kind] = {
                "nt_i": sb(f"nt_i_{kind}", [NS, 1], i32),
                "ss_i": sb(f"ss_i_{kind}", [NS, K], i32),
                "se_i": sb(f"se_i_{kind}", [NS, K], i32),
                "ntf": sb(f"ntf_{kind}", [NS, 1], f32),
                "ssf": sb(f"ssf_{kind}", [NS, K], f32),
                "sep1": sb(f"sep1_{kind}", [NS, K], f32),
                "sep1f": sb(f"sep1f_{kind}", [NS, K], f32),
                "w_pt": sb(f"w_pt_{kind}", [NS, L], f32),
                "w_sent": sb(f"w_sent_{kind}", [NS, L], f32),
                "wT": sb(f"wT_{kind}", [128, 2 * NS + 32], mmdt),
            }
        # shared DVE scratch
        sm = sb("sm", [NS, L], f32)
        spm = sb("spm", [NS, L], f32)
        tmp = sb("tmp", [NS, L], f32)
        cnt_s = sb("cnt_s", [NS, 1], f32)
        cnt_p = sb("cnt_p", [NS, 1], f32)
        has = sb("has", [NS, 1], f32)
        cntp_c = sb("cntp_c", [NS, 1], f32)
        recip_p = sb("recip_p", [NS, 1], f32)
        recip_s = sb("recip_s", [NS, 1], f32)
        rp_sel = sb("rp_sel", [NS, 1], f32)
        nh = sb("nh", [NS, 1], f32)
        rs_sel = sb("rs_sel", [NS, 1], f32)

        ft = [sb(f"ft{i}", [128, 4, H], mmdt) for i in range(NBUF)]
        stage = [sb(f"stage{i}", [128, G8, H], f32) for i in range(2)]
        psA = [ps(f"psA{i}", [128, 512], f32) for i in range(2)]
        psB = [ps(f"psB{i}", [128, 512], f32) for i in range(2)]

        const_sem = sem("const_sem")   # iota/ident DMAs         (SP -> DVE/PE)
        wdma_sem = sem("wdma_sem")    # nt/ss/se DMAs           (SP -> DVE)
        dve_sem = sem("dve_sem")     # per-kind weights ready  (DVE -> PE)
        pe_w_sem = sem("pe_w_sem")    # per-kind transposes     (PE -> ACT)
        wact_sem = sem("wact_sem")    # per-kind wT interleave  (ACT -> PE)
        ft_sem = [sem(f"ft_sem{i}") for i in range(NBUF)]  # feat DMAs (SP -> PE)
        pe_grp = sem("pe_grp")      # per-group matmuls done  (PE -> ACT, SP)
        act_grp = sem("act_grp")     # per-group drains done   (ACT -> PE)
        odma_sem = [sem(f"odma_sem{i}") for i in range(2)]  # out DMAs per stage buf
        dve_chain = sem("dve_chain")  # same-engine RAW ordering on DVE
        act_chain = sem("act_chain")  # ACT copy -> ACT-issued DMA ordering

        with nc.Block() as block:

            @block.sync
            def _(sync):
                sync.dma_start(out=iota_span[:], in_=iota_span_d[:, :]).then_inc(const_sem, 16)
                sync.dma_start(out=iota_sent[:], in_=iota_sent_d[:, :]).then_inc(const_sem, 16)
                sync.dma_start(out=ident[:], in_=ident_d[:, :]).then_inc(const_sem, 16)
                for kind in kinds:
                    w = wtiles[kind]
                    sync.dma_start(out=w["nt_i"][:], in_=ins[f"{kind}_nt"][:].unsqueeze(1)).then_inc(wdma_sem, 16)
                    sync.dma_start(out=w["ss_i"][:], in_=ins[f"{kind}_ss"][:, :]).then_inc(wdma_sem, 16)
                    sync.dma_start(out=w["se_i"][:], in_=ins[f"{kind}_se"][:, :]).then_inc(wdma_sem, 16)
                # main loop: feature tile loads
                for rep in range(repeat):
                  for kidx, kind in enumerate(kinds):
                    feat = ins[f"{kind}_feat"]
                    for g in range(NGRP):
                        gg = (rep * 2 + kidx) * NGRP + g
                        if gg >= NBUF:
                            sync.wait_ge(pe_grp, gg - NBUF + 1)
                        sync.dma_start(
                            out=ft[gg % NBUF][:],
                            in_=feat[4 * g:4 * g + 4, :, :].transpose([1, 0, 2]),
                        ).then_inc(ft_sem[gg % NBUF], 16)

            @block.vector
            def _(vector):
                # The DVE pipeline has no interlock for back-to-back RAW, so
                # chain every op through a self-semaphore.
                nv = [0]

                def dv(res):
                    res.then_inc(dve_chain, 1)
                    nv[0] += 1

                def dw():
                    if nv[0]:
                        vector.wait_ge(dve_chain, nv[0])

                vector.wait_ge(const_sem, 48)
                for kidx, kind in enumerate(kinds):
                    w = wtiles[kind]
                    if kidx == 0:
                        vector.wait_ge(wdma_sem, 96)
                    dv(vector.tensor_copy(out=w["ntf"][:], in_=w["nt_i"][:]))
                    dv(vector.tensor_copy(out=w["ssf"][:], in_=w["ss_i"][:]))
                    dv(vector.tensor_scalar(
                        out=w["sep1"][:], in0=w["se_i"][:], scalar1=1,
                        scalar2=None, op0=Alu.add))
                    dw()
                    dv(vector.tensor_copy(out=w["sep1f"][:], in_=w["sep1"][:]))
                    # sentence mask + count
                    dw()
                    dv(vector.tensor_scalar(
                        out=sm[:], in0=iota_sent[:], scalar1=w["ntf"][:],
                        scalar2=0.0, op0=Alu.is_le, op1=Alu.add,
                        accum_out=cnt_s[:]))
                    # span union mask: sum_k [ge(l, s_k) - ge(l, e_k + 1)]
                    dv(vector.tensor_scalar(
                        out=spm[:], in0=iota_span[:], scalar1=w["ssf"][:, 0:1],
                        scalar2=None, op0=Alu.is_ge))
                    for k in range(1, K):
                        dw()
                        dv(vector.tensor_scalar(
                            out=tmp[:], in0=iota_span[:],
                            scalar1=w["ssf"][:, k:k + 1],
                            scalar2=None, op0=Alu.is_ge))
                        dw()
                        dv(vector.tensor_tensor(out=spm[:], in0=spm[:], in1=tmp[:], op=Alu.add))
                    for k in range(K):
                        dw()
                        dv(vector.tensor_scalar(
                            out=tmp[:], in0=iota_span[:],
                            scalar1=w["sep1f"][:, k:k + 1],
                            scalar2=None, op0=Alu.is_ge))
                        dw()
                        dv(vector.tensor_tensor(out=spm[:], in0=spm[:], in1=tmp[:], op=Alu.subtract))
                    dw()
                    dv(vector.reduce_sum(out=cnt_p[:], in_=spm[:], axis=mybir.AxisListType.X))
                    dw()
                    dv(vector.tensor_scalar(
                        out=has[:], in0=cnt_p[:], scalar1=1.0, scalar2=None, op0=Alu.is_ge))
                    dv(vector.tensor_scalar(
                        out=cntp_c[:], in0=cnt_p[:], scalar1=1.0, scalar2=None, op0=Alu.max))
                    dw()
                    dv(vector.reciprocal(out=recip_p[:], in_=cntp_c[:]))
                    dv(vector.reciprocal(out=recip_s[:], in_=cnt_s[:]))
                    dw()
                    dv(vector.tensor_tensor(out=rp_sel[:], in0=has[:], in1=recip_p[:], op=Alu.mult))
                    dv(vector.tensor_scalar(
                        out=nh[:], in0=has[:], scalar1=-1.0, scalar2=1.0,
                        op0=Alu.mult, op1=Alu.add))
                    dw()
                    dv(vector.tensor_tensor(out=rs_sel[:], in0=nh[:], in1=recip_s[:], op=Alu.mult))
                    dv(vector.tensor_scalar(
                        out=w["w_sent"][:], in0=sm[:], scalar1=recip_s[:],
                        scalar2=None, op0=Alu.mult))
                    dv(vector.tensor_scalar(
                        out=w["w_pt"][:], in0=spm[:], scalar1=rp_sel[:],
                        scalar2=None, op0=Alu.mult))
                    dw()
                    dv(vector.tensor_scalar(
                        out=tmp[:], in0=sm[:], scalar1=rs_sel[:],
                        scalar2=None, op0=Alu.mult))
                    dw()
                    dv(vector.tensor_tensor(out=w["w_pt"][:], in0=w["w_pt"][:], in1=tmp[:], op=Alu.add))
                    # zero pad columns (so M=32 matmuls can slide past the end)
                    dw()
                    vector.memset(w["wT"][:, 2 * NS:], 0.0).then_inc(dve_sem, 1)

            @block.tensor
            def _(tensor):
                # weight transposes [n, L] -> [L, n], into main-loop PSUM banks
                for kidx, kind in enumerate(kinds):
                    w = wtiles[kind]
                    tensor.wait_ge(dve_sem, kidx + 1)
                    tensor.transpose(psA[kidx][:, 0:128], w["w_pt"][:], ident[:])
                    tensor.transpose(
                        psB[kidx][:, 0:128], w["w_sent"][:], ident[:]
                    ).then_inc(pe_w_sem, 1)
                # main loop
                tensor.wait_ge(wact_sem, 2)
                for rep in range(repeat):
                  for kidx, kind in enumerate(kinds):
                    wT = wtiles[kind]["wT"]
                    for g in range(NGRP):
                        gg = (rep * 2 + kidx) * NGRP + g
                        pb = gg % 2
                        tensor.wait_ge(ft_sem[gg % NBUF], 16 * (gg // NBUF + 1))
                        if gg >= 2:
                            tensor.wait_ge(act_grp, gg - 1)
                        for j in range(4):
                            n_ = 4 * g + j
                            lhsT = wT[:, 2 * n_:2 * n_ + 32]
                            tensor.matmul(
                                out=psA[pb][32 * j:32 * j + 32, :], lhsT=lhsT,
                                rhs=ft[gg % NBUF][:, j, 0:512],
                                start=True, stop=True, tile_position=(0, 32 * j))
                            mm = tensor.matmul(
                                out=psB[pb][32 * j:32 * j + 32, 0:256], lhsT=lhsT,
                                rhs=ft[gg % NBUF][:, j, 512:H],
                                start=True, stop=True, tile_position=(0, 32 * j))
                        mm.then_inc(pe_grp, 1)

            @block.scalar
            def _(scalar):
                # wT interleave: even cols = pt weights, odd cols = sent weights
                for kidx, kind in enumerate(kinds):
                    w = wtiles[kind]
                    wT_v = w["wT"][:, 0:2 * NS].rearrange("p (n t) -> p t n", t=2)
                    scalar.wait_ge(pe_w_sem, kidx + 1)
                    scalar.copy(out=wT_v[:, 0, :], in_=psA[kidx][:, 0:128])
                    cp = scalar.copy(out=wT_v[:, 1, :], in_=psB[kidx][:, 0:128])
                    cp.then_inc(wact_sem, 1)
                # main loop: drains + output DMAs
                n_odma = 0
                for rep in range(repeat):
                  for kidx, kind in enumerate(kinds):
                    pt_out = outs[f"{kind}_pt"]
                    sent_out = outs[f"{kind}_sent"]
                    for g in range(NGRP):
                        gg = (rep * 2 + kidx) * NGRP + g
                        pb = gg % 2
                        sgg = gg // G8
                        st = stage[sgg % 2]
                        if g % G8 == 0 and sgg >= 2:
                            scalar.wait_ge(odma_sem[sgg % 2], 128 * (sgg // 2))
                        scalar.wait_ge(pe_grp, gg + 1)
                        scalar.copy(out=st[:, g % G8, 0:512], in_=psA[pb][:])
                        cp = scalar.copy(out=st[:, g % G8, 512:H], in_=psB[pb][:, 0:256])
                        cp.then_inc(act_grp, 1)
                        if g % G8 == G8 - 1:
                            # ensure the staging writes retired before the
                            # DMA engines read them
                            scalar.wait_ge(act_grp, gg + 1)
                            base = 4 * G8 * (g // G8)
                            for j in range(4):
                                scalar.dma_start(
                                    out=pt_out[base + j:base + 4 * G8:4, :].unsqueeze(0),
                                    in_=st[32 * j:32 * j + 1, :, :],
                                ).then_inc(odma_sem[sgg % 2], 16)
                                scalar.dma_start(
                                    out=sent_out[base + j:base + 4 * G8:4, :].unsqueeze(0),
                                    in_=st[32 * j + 1:32 * j + 2, :, :],
                                ).then_inc(odma_sem[sgg % 2], 16)
                                n_odma += 2
                total_sg = 2 * NSG * repeat
                scalar.wait_ge(odma_sem[0], 128 * ((total_sg + 1) // 2))
                scalar.wait_ge(odma_sem[1], 128 * (total_sg // 2))

    return nc


def _get_nc():
    if "nc" not in _CACHE:
        _CACHE["nc"] = _build_nc()
    return _CACHE["nc"]


def _make_in_maps(inputs):
    def npa(x, dt):
        return np.ascontiguousarray(np.asarray(x), dtype=dt)

    full = {
        "rv_feat": npa(inputs["review_feat"], np.float32),
        "rp_feat": npa(inputs["reply_feat"], np.float32),
        "rv_nt": npa(inputs["review_num_tokens"], np.int32),
        "rp_nt": npa(inputs["reply_num_tokens"], np.int32),
        "rv_ss": npa(inputs["review_span_start"], np.int32),
        "rv_se": npa(inputs["review_span_end"], np.int32),
        "rp_ss": npa(inputs["reply_span_start"], np.int32),
        "rp_se": npa(inputs["reply_span_end"], np.int32),
    }
    in_maps = []
    for c in range(NCORES):
        sl = slice(c * NS, (c + 1) * NS)
        in_maps.append({k: v[sl] for k, v in full.items()})
    return in_maps


def _gather(results):
    def cat(name):
        return np.concatenate([results[c][name] for c in range(NCORES)], axis=0)

    return cat("rv_pt"), cat("rv_sent"), cat("rp_pt"), cat("rp_sent")


def kernel(**inputs):
    from concourse.bass_utils import run_bass_kernel_spmd

    nc = _get_nc()
    in_maps = _make_in_maps(inputs)
    res = run_bass_kernel_spmd(nc, in_maps, list(range(NCORES)))
    return _gather(res.results)


# revision 20
# speedup vs baseline: 963.2691x; 2.1369x over previous
# Trainium2 Bass kernel for nn_BertProber (segment_reduce, memory-bound).
#
# Sharding: pure data parallel over the sentence dim N=1024 -> 8 cores x 128
# sentences. Each core processes its 128 review + 128 reply sentences
# independently; no collectives.
#
# Per-core algorithm:
#   For each sentence n the reference computes two weighted token means over
#   the [L=128, H=768] feature tile: a sentence mean (tokens 1..nt) and a
#   probing-topic mean (union of <=5 spans, falling back to the sentence mean
#   when no span is valid). Both are expressible as  w.T @ feat  with a
#   per-sentence weight vector w[L] that already folds in the 1/count scaling
#   and the has_span selection:
#     1. Build all weight vectors on-chip in [n_part, L_free] layout with DVE
#        compare/accumulate ops on an iota constant (span union via
#        difference-of-step-functions: sum_k ge(l,s_k) - ge(l,e_k+1)).
#     2. Transpose to [L, n] via PE and interleave (pt, sent) columns.
#     3. Main loop: DMA feature tiles [128L, 4 sentences x 768H]; per sentence
#        two matmuls (H split 512+256) with lhsT = 32 interleaved weight
#        columns so results land at PSUM partitions {32j, 32j+1} and every
#        PSUM partition is written (M doesn't affect PE cost).
#     4. Drain PSUM -> SBUF staging on the Scalar engine; single-partition
#        DMAs scatter the pt/sent rows to DRAM.
#
# Written in raw bass (manual semaphores, standalone wait instructions): the
# Tile layer's generated code (multi-wait instructions, RANGE_CLEAR cleanup)
# does not compile with this container's walrus build.
import numpy as np

N, L, H, K = 1024, 128, 768, 5
NCORES = 8
NS = N // NCORES   # sentences per core
G8 = 8             # 4-sentence groups per staging supertile (32 sentences)
NBUF = 4           # feature-tile double buffering depth
NGRP = NS // 4     # groups per kind
NSG = NGRP // G8   # supertiles per kind

# "float32" = exact, 4 PE cycles/row. "float32r" = reduced precision, 1 cyc/row.
MM_DTYPE = "bfloat16"

_CACHE = {}


def _build_nc(repeat=1, swap_last=False):
    import concourse.bass as bass
    import concourse.mybir as mybir
    from contextlib import ExitStack

    f32 = mybir.dt.float32
    i32 = mybir.dt.int32
    mmdt = getattr(mybir.dt, MM_DTYPE)
    Alu = mybir.AluOpType

    nc = bass.Bass(trn_type="TRN2")

    kinds = ("rv", "rp")
    ins = {}
    outs = {}
    for kind in kinds:
        ins[f"{kind}_feat"] = nc.dram_tensor(
            f"{kind}_feat", [NS, L, H], f32, kind="ExternalInput")
        ins[f"{kind}_nt"] = nc.dram_tensor(
            f"{kind}_nt", [NS], i32, kind="ExternalInput")
        ins[f"{kind}_ss"] = nc.dram_tensor(
            f"{kind}_ss", [NS, K], i32, kind="ExternalInput")
        ins[f"{kind}_se"] = nc.dram_tensor(
            f"{kind}_se", [NS, K], i32, kind="ExternalInput")
        outs[f"{kind}_pt"] = nc.dram_tensor(
            f"{kind}_pt", [NS, H], f32, kind="ExternalOutput")
        outs[f"{kind}_sent"] = nc.dram_tensor(
            f"{kind}_sent", [NS, H], f32, kind="ExternalOutput")

    # Compile-time constants.
    iota_row = np.tile(np.arange(L, dtype=np.float32)[None, :], (128, 1))
    iota_sent_np = iota_row.copy()
    iota_sent_np[:, 0] = 1000.0  # position 0 ([CLS]) never in the sentence mask
    ident_np = np.eye(128, dtype=np.float32)
    iota_span_d = nc.inline_tensor(iota_row, name="iota_span_c")
    iota_sent_d = nc.inline_tensor(iota_sent_np, name="iota_sent_c")
    ident_d = nc.inline_tensor(ident_np, name="ident_c")

    with ExitStack() as ctx:
        def sb(name, shape, dt):
            return ctx.enter_context(nc.sbuf_tensor(name, shape, dt))

        def ps(name, shape, dt):
            return ctx.enter_context(nc.psum_tensor(name, shape, dt))

        def sem(name):
            return ctx.enter_context(nc.semaphore(name))

        iota_span = sb("iota_span", [128, L], f32)
        iota_sent = sb("iota_sent", [128, L], f32)
        ident = sb("ident", [128, 128], f32)
        # per-kind weight-phase tiles
        wtiles = {}
        for kind in kinds:
            wtiles[kind] = {
                "nt_i": sb(f"nt_i_{kind}", [NS, 1], i32),
                "ss_i": sb(f"ss_i_{kind}", [NS, K], i32),
                "se_i": sb(f"se_i_{kind}", [NS, K], i32),
                "ntf": sb(f"ntf_{kind}", [NS, 1], f32),
                "ssf": sb(f"ssf_{kind}", [NS, K], f32),
                "sep1": sb(f"sep1_{kind}", [NS, K], f32),
                "sep1f": sb(f"sep1f_{kind}", [NS, K], f32),
                "w_pt": sb(f"w_pt_{kind}", [NS, L], f32),
                "w_sent": sb(f"w_sent_{kind}", [NS, L], f32),
                "wT": sb(f"wT_{kind}", [128, 2 * NS + 32], mmdt),
            }
        # shared DVE scratch
        sm = sb("sm", [NS, L], f32)
        spm = sb("spm", [NS, L], f32)
        tmp = sb("tmp", [NS, L], f32)
        cnt_s = sb("cnt_s", [NS, 1], f32)
        cnt_p = sb("cnt_p", [NS, 1], f32)
        has = sb("has", [NS, 1], f32)
        cntp_c = sb("cntp_c", [NS, 1], f32)
        recip_p = sb("recip_p", [NS, 1], f32)
        recip_s = sb("recip_s", [NS, 1], f32)
        rp_sel = sb("rp_sel", [NS, 1], f32)
        nh = sb("nh", [NS, 1], f32)
        rs_sel = sb("rs_sel", [NS, 1], f32)

        ft = [sb(f"ft{i}", [128, 4, H], mmdt) for i in range(NBUF)]
        stage = [sb(f"stage{i}", [128, G8, H], f32) for i in range(2)]
        psA = [ps(f"psA{i}", [128, 512], f32) for i in range(2)]
        psB = [ps(f"psB{i}", [128, 512], f32) for i in range(2)]

        const_sem = sem("const_sem")   # iota/ident DMAs         (SP -> DVE/PE)
        wdma_sem = sem("wdma_sem")    # nt/ss/se DMAs           (SP -> DVE)
        dve_sem = sem("dve_sem")     # per-kind weights ready  (DVE -> PE)
        pe_w_sem = sem("pe_w_sem")    # per-kind transposes     (PE -> ACT)
        wact_sem = sem("wact_sem")    # per-kind wT interleave  (ACT -> PE)
        ft_sem = [sem(f"ft_sem{i}") for i in range(NBUF)]  # feat DMAs (SP -> PE)
        pe_grp = sem("pe_grp")      # per-group matmuls done  (PE -> ACT, SP)
        act_grp = sem("act_grp")     # per-group drains done   (ACT -> PE)
        odma_sem = [sem(f"odma_sem{i}") for i in range(2)]  # out DMAs per stage buf
        dve_chain = sem("dve_chain")  # same-engine RAW ordering on DVE
        act_chain = sem("act_chain")  # ACT copy -> ACT-issued DMA ordering

        with nc.Block() as block:

            @block.sync
            def _(sync):
                sync.dma_start(out=iota_span[:], in_=iota_span_d[:, :]).then_inc(const_sem, 16)
                sync.dma_start(out=iota_sent[:], in_=iota_sent_d[:, :]).then_inc(const_sem, 16)
                sync.dma_start(out=ident[:], in_=ident_d[:, :]).then_inc(const_sem, 16)
                for kind in kinds:
                    w = wtiles[kind]
                    sync.dma_start(out=w["nt_i"][:], in_=ins[f"{kind}_nt"][:].unsqueeze(1)).then_inc(wdma_sem, 16)
                    sync.dma_start(out=w["ss_i"][:], in_=ins[f"{kind}_ss"][:, :]).then_inc(wdma_sem, 16)
                    sync.dma_start(out=w["se_i"][:], in_=ins[f"{kind}_se"][:, :]).then_inc(wdma_sem, 16)
            @block.gpsimd
            def _(gpsimd):
                # feature tile loads: SWDGE casts fp32 -> bf16 in flight
                for rep in range(repeat):
                  for kidx, kind in enumerate(kinds):
                    feat = ins[f"{kind}_feat"]
                    for g in range(NGRP):
                        gg = (rep * 2 + kidx) * NGRP + g
                        if gg >= NBUF:
                            gpsimd.wait_ge(pe_grp, gg - NBUF + 1)
                        gpsimd.dma_start(
                            out=ft[gg % NBUF][:],
                            in_=feat[4 * g:4 * g + 4, :, :].transpose([1, 0, 2]),
                        ).then_inc(ft_sem[gg % NBUF], 16)

            @block.vector
            def _(vector):
                # The DVE pipeline has no interlock for back-to-back RAW, so
                # chain every op through a self-semaphore.
                nv = [0]

                def dv(res):
                    res.then_inc(dve_chain, 1)
                    nv[0] += 1

                def dw():
                    if nv[0]:
                        vector.wait_ge(dve_chain, nv[0])

                vector.wait_ge(const_sem, 48)
                for kidx, kind in enumerate(kinds):
                    w = wtiles[kind]
                    if kidx == 0:
                        vector.wait_ge(wdma_sem, 96)
                    dv(vector.tensor_copy(out=w["ntf"][:], in_=w["nt_i"][:]))
                    dv(vector.tensor_copy(out=w["ssf"][:], in_=w["ss_i"][:]))
                    dv(vector.tensor_scalar(
                        out=w["sep1"][:], in0=w["se_i"][:], scalar1=1,
                        scalar2=None, op0=Alu.add))
                    dw()
                    dv(vector.tensor_copy(out=w["sep1f"][:], in_=w["sep1"][:]))
                    # sentence mask + count
                    dw()
                    dv(vector.tensor_scalar(
                        out=sm[:], in0=iota_sent[:], scalar1=w["ntf"][:],
                        scalar2=0.0, op0=Alu.is_le, op1=Alu.add,
                        accum_out=cnt_s[:]))
                    # span union mask: sum_k [ge(l, s_k) - ge(l, e_k + 1)]
                    dv(vector.tensor_scalar(
                        out=spm[:], in0=iota_span[:], scalar1=w["ssf"][:, 0:1],
                        scalar2=None, op0=Alu.is_ge))
                    for k in range(1, K):
                        dw()
                        dv(vector.tensor_scalar(
                            out=tmp[:], in0=iota_span[:],
                            scalar1=w["ssf"][:, k:k + 1],
                            scalar2=None, op0=Alu.is_ge))
                        dw()
                        dv(vector.tensor_tensor(out=spm[:], in0=spm[:], in1=tmp[:], op=Alu.add))
                    for k in range(K):
                        dw()
                        dv(vector.tensor_scalar(
                            out=tmp[:], in0=iota_span[:],
                            scalar1=w["sep1f"][:, k:k + 1],
                            scalar2=None, op0=Alu.is_ge))
                        dw()
                        dv(vector.tensor_tensor(out=spm[:], in0=spm[:], in1=tmp[:], op=Alu.subtract))
                    dw()
                    dv(vector.reduce_sum(out=cnt_p[:], in_=spm[:], axis=mybir.AxisListType.X))
                    dw()
                    dv(vector.tensor_scalar(
                        out=has[:], in0=cnt_p[:], scalar1=1.0, scalar2=None, op0=Alu.is_ge))
                    dv(vector.tensor_scalar(
                        out=cntp_c[:], in0=cnt_p[:], scalar1=1.0, scalar2=None, op0=Alu.max))
                    dw()
                    dv(vector.reciprocal(out=recip_p[:], in_=cntp_c[:]))
                    dv(vector.reciprocal(out=recip_s[:], in_=cnt_s[:]))
                    dw()
                    dv(vector.tensor_tensor(out=rp_sel[:], in0=has[:], in1=recip_p[:], op=Alu.mult))
                    dv(vector.tensor_scalar(
                        out=nh[:], in0=has[:], scalar1=-1.0, scalar2=1.0,
                        op0=Alu.mult, op1=Alu.add))
                    dw()
                    dv(vector.tensor_tensor(out=rs_sel[:], in0=nh[:], in1=recip_s[:], op=Alu.mult))
                    dv(vector.tensor_scalar(
                        out=w["w_sent"][:], in0=sm[:], scalar1=recip_s[:],
                        scalar2=None, op0=Alu.mult))
                    dv(vector.tensor_scalar(
                        out=w["w_pt"][:], in0=spm[:], scalar1=rp_sel[:],
                        scalar2=None, op0=Alu.mult))
                    dw()
                    dv(vector.tensor_scalar(
                        out=tmp[:], in0=sm[:], scalar1=rs_sel[:],
                        scalar2=None, op0=Alu.mult))
                    dw()
                    dv(vector.tensor_tensor(out=w["w_pt"][:], in0=w["w_pt"][:], in1=tmp[:], op=Alu.add))
                    # zero pad columns (so M=32 matmuls can slide past the end)
                    dw()
                    vector.memset(w["wT"][:, 2 * NS:], 0.0).then_inc(dve_sem, 1)

            @block.tensor
            def _(tensor):
                # weight transposes [n, L] -> [L, n], into main-loop PSUM banks
                for kidx, kind in enumerate(kinds):
                    w = wtiles[kind]
                    tensor.wait_ge(dve_sem, kidx + 1)
                    tensor.transpose(psA[kidx][:, 0:128], w["w_pt"][:], ident[:])
                    tensor.transpose(
                        psB[kidx][:, 0:128], w["w_sent"][:], ident[:]
                    ).then_inc(pe_w_sem, 1)
                # main loop
                tensor.wait_ge(wact_sem, 2)
                for rep in range(repeat):
                  for kidx, kind in enumerate(kinds):
                    wT = wtiles[kind]["wT"]
                    for g in range(NGRP):
                        gg = (rep * 2 + kidx) * NGRP + g
                        pb = gg % 2
                        tensor.wait_ge(ft_sem[gg % NBUF], 16 * (gg // NBUF + 1))
                        if gg >= 2:
                            tensor.wait_ge(act_grp, gg - 1)
                        for j in range(4):
                            n_ = 4 * g + j
                            lhsT = wT[:, 2 * n_:2 * n_ + 32]
                            tensor.matmul(
                                out=psA[pb][32 * j:32 * j + 32, :], lhsT=lhsT,
                                rhs=ft[gg % NBUF][:, j, 0:512],
                                start=True, stop=True, tile_position=(0, 32 * j))
                            mm = tensor.matmul(
                                out=psB[pb][32 * j:32 * j + 32, 0:256], lhsT=lhsT,
                                rhs=ft[gg % NBUF][:, j, 512:H],
                                start=True, stop=True, tile_position=(0, 32 * j))
                        mm.then_inc(pe_grp, 1)

            @block.scalar
            def _(scalar):
                # wT interleave: even cols = pt weights, odd cols = sent weights
                for kidx, kind in enumerate(kinds):
                    w = wtiles[kind]
                    wT_v = w["wT"][:, 0:2 * NS].rearrange("p (n t) -> p t n", t=2)
                    scalar.wait_ge(pe_w_sem, kidx + 1)
                    scalar.copy(out=wT_v[:, 0, :], in_=psA[kidx][:, 0:128])
                    cp = scalar.copy(out=wT_v[:, 1, :], in_=psB[kidx][:, 0:128])
                    cp.then_inc(wact_sem, 1)
                # main loop: drains + output DMAs
                n_odma = 0
                for rep in range(repeat):
                  for kidx, kind in enumerate(kinds):
                    pt_out = outs[f"{kind}_pt"]
                    sent_out = outs[f"{kind}_sent"]
                    for g in range(NGRP):
                        gg = (rep * 2 + kidx) * NGRP + g
                        pb = gg % 2
                        sgg = gg // G8
                        st = stage[sgg % 2]
                        if g % G8 == 0 and sgg >= 2:
                            scalar.wait_ge(odma_sem[sgg % 2], 128 * (sgg // 2))
                        scalar.wait_ge(pe_grp, gg + 1)
                        scalar.copy(out=st[:, g % G8, 0:512], in_=psA[pb][:])
                        cp = scalar.copy(out=st[:, g % G8, 512:H], in_=psB[pb][:, 0:256])
                        cp.then_inc(act_grp, 1)
                        if g % G8 == G8 - 1:
                            # ensure the staging writes retired before the
                            # DMA engines read them
                            scalar.wait_ge(act_grp, gg + 1)
                            base = 4 * G8 * (g // G8)
                            o1, o2 = pt_out, sent_out
                            if swap_last and rep == repeat - 1:
                                o1, o2 = sent_out, pt_out
                            for j in range(4):
                                scalar.dma_start(
                                    out=o1[base + j:base + 4 * G8:4, :].unsqueeze(0),
                                    in_=st[32 * j:32 * j + 1, :, :],
                                ).then_inc(odma_sem[sgg % 2], 16)
                                scalar.dma_start(
                                    out=o2[base + j:base + 4 * G8:4, :].unsqueeze(0),
                                    in_=st[32 * j + 1:32 * j + 2, :, :],
                                ).then_inc(odma_sem[sgg % 2], 16)
                                n_odma += 2
                total_sg = 2 * NSG * repeat
                scalar.wait_ge(odma_sem[0], 128 * ((total_sg + 1) // 2))
                scalar.wait_ge(odma_sem[1], 128 * (total_sg // 2))

    return nc


def _get_nc():
    if "nc" not in _CACHE:
        _CACHE["nc"] = _build_nc()
    return _CACHE["nc"]


def _make_in_maps(inputs):
    def npa(x, dt):
        return np.ascontiguousarray(np.asarray(x), dtype=dt)

    full = {
        "rv_feat": npa(inputs["review_feat"], np.float32),
        "rp_feat": npa(inputs["reply_feat"], np.float32),
        "rv_nt": npa(inputs["review_num_tokens"], np.int32),
        "rp_nt": npa(inputs["reply_num_tokens"], np.int32),
        "rv_ss": npa(inputs["review_span_start"], np.int32),
        "rv_se": npa(inputs["review_span_end"], np.int32),
        "rp_ss": npa(inputs["reply_span_start"], np.int32),
        "rp_se": npa(inputs["reply_span_end"], np.int32),
    }
    in_maps = []
    for c in range(NCORES):
        sl = slice(c * NS, (c + 1) * NS)
        in_maps.append({k: v[sl] for k, v in full.items()})
    return in_maps


def _gather(results):
    def cat(name):
        return np.concatenate([results[c][name] for c in range(NCORES)], axis=0)

    return cat("rv_pt"), cat("rv_sent"), cat("rp_pt"), cat("rp_sent")


def kernel(**inputs):
    from concourse.bass_utils import run_bass_kernel_spmd

    nc = _get_nc()
    in_maps = _make_in_maps(inputs)
    res = run_bass_kernel_spmd(nc, in_maps, list(range(NCORES)))
    return _gather(res.results)
